# revision 1
# baseline (speedup 1.0000x reference)
"""TRN2 Bass kernel for nn_BalancedHamiltonLayer.

Math: out[n,k,j] = sum_{r,s,i} x[n,s,i] * factors_B[r,j,i] * H(A)[r,k,s] + bias
collapses to a single dense matmul  out = x2d @ W + bias  with
W[(s,i),(k,j)] = sum_r H[r,k,s] * B[r,j,i]  (a 1024x1024 matrix folded on host
in float64).

Sharding: data-parallel over the 8192 token rows across 8 NeuronCores
(1024 rows each); W replicated.  The matmul runs in fp16 on the PE
(full-rate, FWL weight loads, ~5e-4 relative error; fp32 PSUM
accumulation).  x is passed pre-transposed per core as
[m2, f_in, 256 tokens] so lhsT tiles load contiguously (512B bursts).
bias is added on the host during the gather.
"""

import numpy as np
import concourse.bacc as bacc
import concourse.mybir as mybir
import concourse.tile as tile
from concourse.bass_utils import run_bass_kernel_spmd

B, T, D = 4, 2048, 1024
RANK, FACTOR, SUB = 8, 64, 4
S = 4 * SUB  # 16
NCORES = 8
NTOK = B * T // NCORES  # 1024 token rows per core
P = 128
KT = D // P     # 8 contraction chunks
MT = NTOK // P  # 8 token tiles per core
M2 = MT // 2    # x DMA granularity: 256-token slabs
NH = 512        # f_out half (one PSUM bank)

_cached_nc = None


def build_module():
    global _cached_nc
    if _cached_nc is not None:
        return _cached_nc
    nc = bacc.Bacc("TRN2", target_bir_lowering=False, debug=False)
    xT = nc.dram_tensor("xT", [M2, D, 2 * P], mybir.dt.float16, kind="ExternalInput").ap()
    w = nc.dram_tensor("w", [D, D], mybir.dt.float16, kind="ExternalInput").ap()
    out = nc.dram_tensor("out", [NTOK, D], mybir.dt.float32, kind="ExternalOutput").ap()

    with tile.TileContext(nc) as tc:
        with (
            tc.tile_pool(name="wp", bufs=1) as wp,
            tc.tile_pool(name="xp", bufs=1) as xp,
            tc.tile_pool(name="op", bufs=4) as op,
            tc.tile_pool(name="ps", bufs=4, space="PSUM") as ps,
        ):
            # PE HAM pre-warm: matmuls on a zeroed SBUF tile accumulate +0
            # into the first real PSUM group while data DMAs are in flight,
            # so the clock gate is at 2.4 GHz when real matmuls start.
            g = xp.tile([P, NH], mybir.dt.float16, tag="warm", name="g")
            nc.gpsimd.memset(g[:], 0.0)

            # All loads on one HWDGE ring (sync), in consumption-deadline
            # order.  SWDGE sem completion is several us late, and parallel
            # rings share SDMA bandwidth packet-round-robin (every transfer
            # then finishes late together) — sequential draining on a
            # single ring gives early slots early completion.
            xt = {}
            wt = {}

            def x_tile(m2, name):
                t = xp.tile([P, KT, 2 * P], mybir.dt.float16, tag=f"x{m2}", name=name)
                xt[m2] = t
                return t

            def w_tile(k, name):
                t = wp.tile([P, 2 * NH], mybir.dt.float16, tag=f"w{k}", name=name)
                wt[k] = t
                return t

            for m2 in range(M2):
                x_tile(m2, f"xt{m2}")
            for k in range(KT):
                w_tile(k, f"wt{k}")

            def xsrc(m2):
                return xT[m2].rearrange("(k p) t -> p k t", p=P)

            # Single ring, strict deadline order: sequential draining means
            # early slots COMPLETE early (parallel rings share SDMA packet
            # round-robin, which makes every transfer finish late together).
            loads = [
                (xt[0][:], xsrc(0)),
                (wt[0][:], w[0:P, :]),
                (xt[1][:], xsrc(1)),
            ] + [
                (wt[k][:], w[k * P:(k + 1) * P, :]) for k in range(1, KT)
            ] + [
                (xt[2][:], xsrc(2)),
                (xt[3][:], xsrc(3)),
            ]
            for da, sa in loads:
                nc.sync.dma_start(da, sa)

            def emit_out(m, pt, halves=False):
                # halves=True pipelines the final tile's copy+store in
                # half-width pieces to shorten the kernel tail.
                nhalf = 2 if halves else 1
                w_piece = NH // nhalf
                for n in range(2):
                    for h in range(nhalf):
                        o = op.tile([P, w_piece], mybir.dt.float32, tag="o", name="o")
                        nc.vector.tensor_copy(
                            o[:], pt[n][:, h * w_piece:(h + 1) * w_piece]
                        )
                        c0 = n * NH + h * w_piece
                        nc.sync.dma_start(
                            out[m * P:(m + 1) * P, c0:c0 + w_piece], o[:]
                        )

            def xs_of(m):
                return xt[m // 2][:, :, (m % 2) * P:(m % 2 + 1) * P]

            with nc.named_scope("mm"):
                # Phase 1: m=0..3 k-interleaved across all 8 PSUM banks —
                # per-k compute (8 MMs, ~1.7us) exceeds the W-chunk arrival
                # cadence, so the PE absorbs DMA-completion jitter without
                # stalling (stalls would also re-throttle the HAM clock).
                NP1 = 4
                pts = {
                    m: {
                        n: ps.tile([P, NH], mybir.dt.float32, tag=f"ps{n}", name=f"pt{m}_{n}")
                        for n in range(2)
                    }
                    for m in range(NP1)
                }
                NWARM = 9
                for i in range(NWARM):
                    nc.tensor.matmul(
                        pts[0][0][:], g[:, :P], g[:], start=(i == 0), stop=False
                    )
                for k in range(KT):
                    for m in range(NP1):
                        for n in range(2):
                            nc.tensor.matmul(
                                pts[m][n][:],
                                xs_of(m)[:, k, :],
                                wt[k][:, n * NH:(n + 1) * NH],
                                start=(k == 0 and not (m == 0 and n == 0)),
                                stop=(k == KT - 1),
                            )
                for m in range(NP1):
                    emit_out(m, pts[m])

                # Phase 2: k-contiguous per m-tile (PE stays warm, dense)
                for m in range(NP1, MT):
                    pt = {
                        n: ps.tile([P, NH], mybir.dt.float32, tag=f"ps{n}", name=f"pt{n}")
                        for n in range(2)
                    }
                    for k in range(KT):
                        for n in range(2):
                            nc.tensor.matmul(
                                pt[n][:],
                                xs_of(m)[:, k, :],
                                wt[k][:, n * NH:(n + 1) * NH],
                                start=(k == 0),
                                stop=(k == KT - 1),
                            )
                    emit_out(m, pt, halves=(m == MT - 1))
    nc.compile()
    _cached_nc = nc
    return nc


def _construct_hamilton(A):
    # A: [rank, 4, sub, sub] -> [rank, 4*sub, 4*sub]
    r, i, j, k = A[:, 0], A[:, 1], A[:, 2], A[:, 3]
    return np.concatenate(
        [
            np.concatenate([r, -i, -j, -k], axis=2),
            np.concatenate([i, r, -k, j], axis=2),
            np.concatenate([j, k, r, -i], axis=2),
            np.concatenate([k, -j, i, r], axis=2),
        ],
        axis=1,
    )


def build_in_maps(x, A, factors_B):
    H = _construct_hamilton(np.asarray(A, dtype=np.float64))  # [r, k, s]
    Bf = np.asarray(factors_B, dtype=np.float64)  # [r, j, i]
    # W[(s,i),(k,j)] = sum_r H[r,k,s] * B[r,j,i]
    W = np.einsum("rks,rji->sikj", H, Bf).reshape(D, D).astype(np.float16)

    x2 = np.asarray(x, dtype=np.float16).reshape(NCORES, NTOK, D)
    in_maps = []
    for c in range(NCORES):
        # [NTOK, D] -> [M2, 256, D] -> [M2, D, 256]
        xs = np.ascontiguousarray(x2[c].reshape(M2, 2 * P, D).transpose(0, 2, 1))
        in_maps.append({"xT": xs, "w": W})
    return in_maps


def kernel(x, A, factors_B, bias):
    nc = build_module()
    in_maps = build_in_maps(x, A, factors_B)
    br = run_bass_kernel_spmd(nc, in_maps, core_ids=list(range(NCORES)))
    out = np.concatenate([r["out"] for r in br.results], axis=0)
    out = out + np.asarray(bias, dtype=np.float32)[None, :]
    return out.reshape(B, T, D).astype(np.float32)



# revision 2
# speedup vs baseline: 1.1342x; 1.1342x over previous
"""TRN2 Bass kernel for nn_BalancedHamiltonLayer.

Math: out[n,k,j] = sum_{r,s,i} x[n,s,i] * factors_B[r,j,i] * H(A)[r,k,s] + bias
collapses to a single dense matmul  out = x2d @ W + bias  with
W[(s,i),(k,j)] = sum_r H[r,k,s] * B[r,j,i]  (a 1024x1024 matrix folded on host
in float64).

Sharding: data-parallel over the 8192 token rows across 8 NeuronCores
(1024 rows each); W replicated.  The matmul runs in fp16 on the PE
(full-rate, FWL weight loads, fp32 PSUM accumulation); the output is
stored as fp16 (adds ~5e-4 relative quantization, tolerance is 2e-2)
to halve store traffic, and upcast + bias-added on the host.

Schedule (per core, derived from NTFF traces of the previous version):
- all tensors are packed on host so every DMA is 2KB/partition contiguous
- loads stream on the sync HWDGE queue in consumption order: x is split
  into 8 one-m-tile (128-token) 256KB transfers interleaved with the 8
  256KB W row-chunks; w0 is issued in parallel on the scalar queue so
  the first real matmul isn't serialized behind two sync-queue issues
- 8 warm-up matmuls on a zeroed tile bridge the DMA latency and get the
  PE HAM clock to 2.4GHz right as real data lands; they accumulate +0
  into the first real PSUM group
- m0,m1 run k-interleaved (4 MMs per W chunk = matches the ~0.85us
  256KB wire cadence); m2..m7 run k-contiguous (everything resident)
- stores go on the scalar HWDGE queue (independent FIFO from loads),
  one 256KB fp16 store per m-tile as soon as its two PSUM halves are
  copied; the last tile's halves are pipelined to shorten the tail
"""

import numpy as np
import concourse.bacc as bacc
import concourse.mybir as mybir
import concourse.tile as tile
from concourse.bass_utils import run_bass_kernel_spmd

B, T, D = 4, 2048, 1024
RANK, FACTOR, SUB = 8, 64, 4
S = 4 * SUB  # 16
NCORES = 8
NTOK = B * T // NCORES  # 1024 token rows per core
P = 128
KT = D // P     # 8 contraction chunks
MT = NTOK // P  # 8 token tiles per core
NH = 512        # f_out half (one PSUM bank)

_cached_nc = None


def build_module():
    global _cached_nc
    if _cached_nc is not None:
        return _cached_nc
    nc = bacc.Bacc("TRN2", target_bir_lowering=False, debug=False)
    xH = nc.dram_tensor("xH", [MT, P, KT, P], mybir.dt.float16, kind="ExternalInput").ap()
    wH = nc.dram_tensor("wH", [KT, P, D], mybir.dt.float16, kind="ExternalInput").ap()
    out = nc.dram_tensor("out", [NTOK, D], mybir.dt.float16, kind="ExternalOutput").ap()

    with tile.TileContext(nc) as tc:
        with (
            tc.tile_pool(name="wp", bufs=1) as wp,
            tc.tile_pool(name="xp", bufs=1) as xp,
            tc.tile_pool(name="op", bufs=1) as op,
            tc.tile_pool(name="ps", bufs=8, space="PSUM") as ps,
        ):
            g = xp.tile([P, NH], mybir.dt.float16, tag="warm", name="g")
            nc.gpsimd.memset(g[:], 0.0)

            xt = {}
            wt = {}
            for m in range(MT):
                xt[m] = xp.tile([P, KT, P], mybir.dt.float16, tag=f"x{m}", name=f"xt{m}")
            for k in range(KT):
                wt[k] = wp.tile([P, D], mybir.dt.float16, tag=f"w{k}", name=f"wt{k}")

            # w0 on the scalar queue: its HWDGE issue (~0.65us) runs in
            # parallel with x0's on sync, so the first matmul's two
            # dependencies arrive together instead of serialized.
            nc.sync.dma_start(xt[0][:], xH[0])
            nc.scalar.dma_start(wt[0][:], wH[0])
            # sync queue drains strictly in this order (~0.85us per 256KB):
            # just-in-time for the matmul schedule below.
            for da, sa in [
                (xt[1], xH[1]),
                (wt[1], wH[1]),
                (wt[2], wH[2]),
                (wt[3], wH[3]),
                (xt[2], xH[2]),
                (wt[4], wH[4]),
                (wt[5], wH[5]),
                (xt[3], xH[3]),
                (wt[6], wH[6]),
                (wt[7], wH[7]),
                (xt[4], xH[4]),
                (xt[5], xH[5]),
                (xt[6], xH[6]),
                (xt[7], xH[7]),
            ]:
                nc.sync.dma_start(da[:], sa)

            ot = {}

            def emit_out(m, pt, pieces=((0, D),)):
                o = op.tile([P, D], mybir.dt.float16, tag=f"o{m}", name=f"o{m}")
                ot[m] = o
                for n in range(2):
                    nc.vector.tensor_copy(o[:, n * NH:(n + 1) * NH], pt[n][:])
                for c0, c1 in pieces:
                    nc.scalar.dma_start(out[m * P:(m + 1) * P, c0:c1], o[:, c0:c1])

            with nc.named_scope("mm"):
                NWARM = 8
                pts = {
                    m: {
                        n: ps.tile([P, NH], mybir.dt.float32, tag="ps", name=f"pt{m}_{n}")
                        for n in range(2)
                    }
                    for m in range(2)
                }
                for i in range(NWARM):
                    nc.tensor.matmul(
                        pts[0][0][:], g[:, :P], g[:], start=(i == 0), stop=False
                    )
                # phase 1: m0,m1 k-interleaved at the W arrival cadence
                for k in range(KT):
                    for m in range(2):
                        for n in range(2):
                            nc.tensor.matmul(
                                pts[m][n][:],
                                xt[m][:, k, :],
                                wt[k][:, n * NH:(n + 1) * NH],
                                start=(k == 0 and not (m == 0 and n == 0)),
                                stop=(k == KT - 1),
                            )
                emit_out(0, pts[0])
                emit_out(1, pts[1])

                # phase 2: k-contiguous per m-tile (all inputs resident)
                for m in range(2, MT):
                    last = m == MT - 1
                    pt = {
                        n: ps.tile([P, NH], mybir.dt.float32, tag="ps", name=f"pt{m}_{n}")
                        for n in range(2)
                    }
                    if last:
                        # stagger the halves so n0's copy+store overlap
                        # n1's matmuls, shortening the kernel tail
                        for n in range(2):
                            for k in range(KT):
                                nc.tensor.matmul(
                                    pt[n][:],
                                    xt[m][:, k, :],
                                    wt[k][:, n * NH:(n + 1) * NH],
                                    start=(k == 0),
                                    stop=(k == KT - 1),
                                )
                            if n == 0:
                                o = op.tile([P, D], mybir.dt.float16, tag=f"o{m}", name=f"o{m}")
                                ot[m] = o
                                nc.vector.tensor_copy(o[:, :NH], pt[0][:])
                                nc.scalar.dma_start(out[m * P:(m + 1) * P, :NH], o[:, :NH])
                        nc.vector.tensor_copy(ot[m][:, NH:], pt[1][:])
                        nc.scalar.dma_start(out[m * P:(m + 1) * P, NH:], ot[m][:, NH:])
                    else:
                        for k in range(KT):
                            for n in range(2):
                                nc.tensor.matmul(
                                    pt[n][:],
                                    xt[m][:, k, :],
                                    wt[k][:, n * NH:(n + 1) * NH],
                                    start=(k == 0),
                                    stop=(k == KT - 1),
                                )
                        emit_out(m, pt)
    nc.compile()
    _cached_nc = nc
    return nc


def _construct_hamilton(A):
    # A: [rank, 4, sub, sub] -> [rank, 4*sub, 4*sub]
    r, i, j, k = A[:, 0], A[:, 1], A[:, 2], A[:, 3]
    return np.concatenate(
        [
            np.concatenate([r, -i, -j, -k], axis=2),
            np.concatenate([i, r, -k, j], axis=2),
            np.concatenate([j, k, r, -i], axis=2),
            np.concatenate([k, -j, i, r], axis=2),
        ],
        axis=1,
    )


def build_in_maps(x, A, factors_B):
    H = _construct_hamilton(np.asarray(A, dtype=np.float64))  # [r, k, s]
    Bf = np.asarray(factors_B, dtype=np.float64)  # [r, j, i]
    # W[(s,i),(k,j)] = sum_r H[r,k,s] * B[r,j,i]
    W = np.einsum("rks,rji->sikj", H, Bf).reshape(D, D).astype(np.float16)
    wH = np.ascontiguousarray(W.reshape(KT, P, D))

    x2 = np.asarray(x, dtype=np.float16).reshape(NCORES, NTOK, D)
    in_maps = []
    for c in range(NCORES):
        # [NTOK, D] -> [m, t, k, p] -> [m, p, k, t] so each per-m DMA is
        # 2KB/partition contiguous
        xs = np.ascontiguousarray(
            x2[c].reshape(MT, P, KT, P).transpose(0, 3, 2, 1)
        )
        in_maps.append({"xH": xs, "wH": wH})
    return in_maps


def kernel(x, A, factors_B, bias):
    nc = build_module()
    in_maps = build_in_maps(x, A, factors_B)
    br = run_bass_kernel_spmd(nc, in_maps, core_ids=list(range(NCORES)))
    out = np.concatenate([r["out"] for r in br.results], axis=0)
    out = out.astype(np.float32) + np.asarray(bias, dtype=np.float32)[None, :]
    return out.reshape(B, T, D)
